# revision 4
# baseline (speedup 1.0000x reference)
"""Trainium2 Bass kernel for nn_Block_9397388444369.

Reference semantics (B=2, T=512, C=256, HID=1024):
    h   = LN(x, g1, b1)
    transform = (h @ Wt.T).reshape(B,T,C,C) * 0.0        # exactly zero
    out = einsum(...) ...                                 # exactly zero
    sa  = 0 @ Wp.T + bp = bp
    x1  = x + bp
    h2  = LN(x1, g2, b2)
    ff  = relu(h2 @ W1.T + bf1) @ W2.T + bf2
    out = x1 + ff

The attention branch collapses to "+bp" (0.0 * finite == 0.0), so the device
only computes the LN + 256->1024->256 MLP. The residual x1, bp and bf2 are
added on the host in fp32 (exact); the device returns just the ff partials.

LayerNorm is folded into the matmuls instead of being a serial pre-pass:
  z[m,t] = rstd[t] * (sum_c y[t,c] w1t[c,m] - mu[t]*s1[m] + sqv[t]*bf1[m])
where s1[m] = sum_c w1t[c,m] (host-computed) and sqv = sqrt(var+eps) =
1/rstd. The "-mu*s1 + sqv*bf1" term is TWO extra contraction rows appended
as a tiny augmented matmul (lhsT = host-packed [s1; bf1], rhs = on-device
[-mu; sqv] transposed stats), so mm1 runs on RAW transposed x (host-packed,
no on-device transpose/normalize) and only the last accumulation step waits
on the stats. Since rstd > 0, relu(rstd*u) = rstd*relu(u): the rstd scale
commutes through the ReLU and is applied once at mm2's output, where t is
the partition dim (per-partition scalar multiply).

Everything is bf16 (inputs, weights, intermediates, output partials), which
halves both DMA bytes and PE cycles (1 cycle/col vs 2 for fp32r). Error is
~1e-3 total on the output, well under the 2e-2 gate.

Sharding: 4 row-groups x 2 HID-halves (like the baseline). DMA plan: two
big input blobs issued from two different queues (Sync HWDGE + GpSimd
SWDGE) so issue cost and data movement overlap, keeping Scalar free for its
activation-table load. Outputs go out per row-tile on Sync/Scalar.
"""

import sys

if '/opt/trn_rl_repo' not in sys.path:
    sys.path.insert(0, '/opt/trn_rl_repo')

import ml_dtypes
import numpy as np

import concourse.bass as bass  # noqa: F401
import concourse.tile as tile
from concourse import bacc, mybir
from concourse.bass_utils import run_bass_kernel_spmd

B, T, C = 2, 512, 256
HID = 4 * C
EPS = 1e-5
N_CORES = 8
N_GROUPS = 4                       # row groups
ROWS = (B * T) // N_GROUPS         # 256 rows per core
RT = ROWS // 128                   # 2 row tiles per core
HH = HID // 2                      # 512-wide hidden half per core
KC = C // 128                      # 2 k-subtiles over C
KH = HH // 128                     # 4 m-chunks over the half

F32 = mybir.dt.float32
BF16 = mybir.dt.bfloat16
BF16_NP = ml_dtypes.bfloat16

# ina blob: xt [128, KC*ROWS] | w1t [128, KC*HH]
INA_XT0 = 0
INA_W10 = KC * ROWS                    # 512
INA_COLS = KC * ROWS + KC * HH         # 1536
# inb blob: xr [128, RT*C] | w2t [128, KH*C] | ident [128, 128]
INB_XR0 = 0
INB_W20 = RT * C                       # 512
INB_ID0 = RT * C + KH * C              # 1536
INB_COLS = INB_ID0 + 128               # 1664


def _build_nc():
    nc = bacc.Bacc("TRN2", target_bir_lowering=False, debug=False,
                   num_devices=N_CORES)

    ina_d = nc.declare_dram_parameter("ina", [128, INA_COLS], BF16,
                                      isOutput=False)
    inb_d = nc.declare_dram_parameter("inb", [128, INB_COLS], BF16,
                                      isOutput=False)
    augw_d = nc.declare_dram_parameter("augw", [2, HH], BF16, isOutput=False)
    y_d = nc.declare_dram_parameter("y", [128, RT * C], BF16, isOutput=True)

    with tile.TileContext(nc) as tc:
        with (
            tc.tile_pool(name="acts", bufs=1) as acts,
            tc.tile_pool(name="stats", bufs=2) as stats,
            tc.tile_pool(name="ptrans", bufs=2, space="PSUM") as ptrans,
            tc.tile_pool(name="pmm1", bufs=4, space="PSUM") as pmm1,
            tc.tile_pool(name="pmm2", bufs=2, space="PSUM") as pmm2,
        ):
            # ---- input DMAs on two independent queues ----
            ina_sb = acts.tile([128, INA_COLS], BF16)
            nc.sync.dma_start(out=ina_sb, in_=ina_d.ap())

            inb_sb = acts.tile([128, INB_COLS], BF16)
            # xr first (gates the stats chain), then weights+identity
            nc.gpsimd.dma_start(out=inb_sb[:, :INB_W20],
                                in_=inb_d.ap()[:, :INB_W20])
            nc.gpsimd.dma_start(out=inb_sb[:, INB_W20:],
                                in_=inb_d.ap()[:, INB_W20:])

            augw_sb = acts.tile([2, HH], BF16)
            nc.sync.dma_start(out=augw_sb, in_=augw_d.ap())

            eps_t = acts.tile([128, 1], F32)
            nc.vector.memset(eps_t, np.float32(EPS))

            ident = inb_sb[:, INB_ID0:INB_ID0 + 128]

            # ---- stats per row tile (DVE + one Scalar sqrt each) ----
            # aug_rhs[0, t] = -mu[t]; aug_rhs[1, t] = sqrt(var[t]+eps)
            aug_rhs = acts.tile([2, ROWS], BF16)
            rstds = []
            stgs = []
            for r in range(RT):
                xr = inb_sb[:, r * C:(r + 1) * C]
                bn6 = stats.tile([128, 6], F32, tag="bn6")
                nc.vector.bn_stats(out=bn6, in_=xr)
                mv = stats.tile([128, 2], F32, tag="mv")
                nc.vector.bn_aggr(out=mv, in_=bn6)
                sqv = stats.tile([128, 1], F32, tag="sqv")
                nc.scalar.activation(out=sqv, in_=mv[:, 1:2],
                                     func=mybir.ActivationFunctionType.Sqrt,
                                     bias=eps_t, scale=1.0)
                rstd = stats.tile([128, 1], F32, tag="rstd")
                nc.vector.reciprocal(out=rstd, in_=sqv)
                rstds.append(rstd)
                stg = stats.tile([128, 2], BF16, tag="stg")
                nc.vector.tensor_scalar_mul(stg[:, 0:1], mv[:, 0:1], -1.0)
                nc.vector.tensor_copy(out=stg[:, 1:2], in_=sqv)
                stgs.append(stg)

            # ---- mm1 raw: ps1[mc] = w1t[:,mc].T @ xt  (PE, gated on ina) ----
            ps1 = []
            for mc in range(KH):
                pf = pmm1.tile([128, ROWS], F32)
                ps1.append(pf)
                for k in range(KC):
                    nc.tensor.matmul(
                        pf,
                        lhsT=ina_sb[:, INA_W10 + k * HH + mc * 128:
                                    INA_W10 + k * HH + mc * 128 + 128],
                        rhs=ina_sb[:, k * ROWS:(k + 1) * ROWS],
                        start=(k == 0), stop=False,
                    )

            # ---- transpose stats into [2, ROWS] (PE, after raw mms) ----
            for r in range(RT):
                pt = ptrans.tile([2, 128], BF16)
                nc.tensor.transpose(pt, stgs[r], ident)
                nc.vector.tensor_copy(out=aug_rhs[:, r * 128:(r + 1) * 128],
                                      in_=pt)

            # ---- aug matmul + relu per m-chunk ----
            relu1 = acts.tile([128, KH, ROWS], BF16)
            for mc in range(KH):
                nc.tensor.matmul(
                    ps1[mc],
                    lhsT=augw_sb[:, mc * 128:(mc + 1) * 128],
                    rhs=aug_rhs,
                    start=False, stop=True,
                )
                nc.scalar.activation(out=relu1[:, mc, :], in_=ps1[mc],
                                     func=mybir.ActivationFunctionType.Relu,
                                     bias=0.0, scale=1.0)

            # ---- mm2 + rstd scale per row tile, then store ----
            y_sb = acts.tile([128, RT, C], BF16)
            for r in range(RT):
                po = pmm2.tile([128, C], F32)
                for mc in range(KH):
                    nc.tensor.matmul(
                        po,
                        lhsT=relu1[:, mc, r * 128:(r + 1) * 128],
                        rhs=inb_sb[:, INB_W20 + mc * C:INB_W20 + (mc + 1) * C],
                        start=(mc == 0), stop=(mc == KH - 1),
                    )
                nc.vector.tensor_scalar_mul(y_sb[:, r, :], po, rstds[r])
                if r == 0:
                    nc.sync.dma_start(out=y_d.ap()[:, :C],
                                      in_=y_sb[:, 0, :])
                else:
                    nc.scalar.dma_start(out=y_d.ap()[:, C:],
                                        in_=y_sb[:, 1, :])

    nc.finalize()
    return nc


_NC_CACHE = None


def _get_nc():
    global _NC_CACHE
    if _NC_CACHE is None:
        _NC_CACHE = _build_nc()
    return _NC_CACHE


def _pack_inputs(x, bp, g2, b2, W1, bf1, W2):
    """Host-side prep: fold bp into x, g2/b2 into W1T/bf1, pack everything
    into the two bf16 SBUF-layout blobs + the tiny aug-weight tensor."""
    x1 = (np.asarray(x, dtype=np.float64).reshape(B * T, C)
          + np.asarray(bp, dtype=np.float64))
    x1_f32 = x1.astype(np.float32)
    x1_bf = x1_f32.astype(BF16_NP)

    w1t_eff = (np.asarray(W1).astype(np.float64).T
               * np.asarray(g2).astype(np.float64)[:, None])      # [C, HID]
    w1t_bf = w1t_eff.astype(np.float32).astype(BF16_NP)
    bf1_eff = (np.asarray(bf1).astype(np.float64)
               + np.asarray(b2).astype(np.float64)
               @ np.asarray(W1).astype(np.float64).T)             # [HID]
    # s1 from the rounded weights actually used on device
    s1 = w1t_bf.astype(np.float64).sum(axis=0)                    # [HID]
    s1_bf = s1.astype(np.float32).astype(BF16_NP)
    bf1_bf = bf1_eff.astype(np.float32).astype(BF16_NP)
    w2t_bf = np.asarray(W2, dtype=np.float32).T.astype(BF16_NP)   # [HID, C]
    ident = np.eye(128, dtype=np.float32).astype(BF16_NP)

    in_maps = []
    for c in range(N_CORES):
        g, hf = c // 2, c % 2
        xg = x1_bf[g * ROWS:(g + 1) * ROWS]                       # [256, C]

        ina = np.empty((128, INA_COLS), dtype=BF16_NP)
        for k in range(KC):
            ina[:, k * ROWS:(k + 1) * ROWS] = xg[:, k * 128:(k + 1) * 128].T
        w1h = w1t_bf[:, hf * HH:(hf + 1) * HH]                    # [C, HH]
        for k in range(KC):
            ina[:, INA_W10 + k * HH:INA_W10 + (k + 1) * HH] = \
                w1h[k * 128:(k + 1) * 128, :]

        inb = np.empty((128, INB_COLS), dtype=BF16_NP)
        for r in range(RT):
            inb[:, r * C:(r + 1) * C] = xg[r * 128:(r + 1) * 128, :]
        w2h = w2t_bf[hf * HH:(hf + 1) * HH]                       # [HH, C]
        for mc in range(KH):
            inb[:, INB_W20 + mc * C:INB_W20 + (mc + 1) * C] = \
                w2h[mc * 128:(mc + 1) * 128, :]
        inb[:, INB_ID0:INB_ID0 + 128] = ident

        augw = np.empty((2, HH), dtype=BF16_NP)
        augw[0] = s1_bf[hf * HH:(hf + 1) * HH]
        augw[1] = bf1_bf[hf * HH:(hf + 1) * HH]

        in_maps.append({"ina": ina, "inb": inb, "augw": augw})
    return in_maps, x1_f32


def _make_in_maps(x, bp, g2, b2, W1, bf1, W2):
    in_maps, _ = _pack_inputs(x, bp, g2, b2, W1, bf1, W2)
    return in_maps


def kernel(x, Wt, Wp, bp, g1, b1, g2, b2, W1, bf1, W2, bf2):
    in_maps, x1_f32 = _pack_inputs(x, bp, g2, b2, W1, bf1, W2)
    nc = _get_nc()
    res = run_bass_kernel_spmd(nc, in_maps, list(range(N_CORES)))

    out = x1_f32.copy()                                       # residual x+bp
    for g in range(N_GROUPS):
        for hf in range(2):
            y = np.asarray(res.results[2 * g + hf]["y"]).astype(np.float32)
            for r in range(RT):
                out[g * ROWS + r * 128:g * ROWS + (r + 1) * 128, :] += \
                    y[:, r * C:(r + 1) * C]
    out = out + np.asarray(bf2, dtype=np.float32)
    return out.reshape(B, T, C).astype(np.float32)


# revision 10
# speedup vs baseline: 1.2196x; 1.2196x over previous
"""Trainium2 Bass kernel for nn_Block_9397388444369.

Reference semantics (B=2, T=512, C=256, HID=1024):
    transform = (h @ Wt.T) * 0.0  -> attention branch is exactly bp
    x1  = x + bp
    ff  = relu(LN(x1,g2,b2) @ W1.T + bf1) @ W2.T + bf2
    out = x1 + ff

Device computes only the MLP partials; x1/bp/bf2 are added on the host in
fp32 (exact). LayerNorm is folded into the matmuls:

  z[m,t] = rstd[t] * (sum_c x1[t,c] w1t[c,m] - mu[t] s1[m] + sigma[t] bf1[m])

The "-mu s1 + sigma bf1" term rides as a 2-row augmented matmul (lhsT =
host-packed [-s1; bf1], rhs = on-device [mu; sigma] obtained by PE-transposing
the bn_stats output), so mm1 runs on RAW host-transposed x and only the last
accumulation waits on stats. rstd > 0 commutes through the ReLU and is
applied once at mm2's fp32 output (t = partition dim there).

mm1/mm2 run in fp8 (e4m3, TRN max +-240) with power-of-2 weight scales
S1=S2=1024 and a 1/16 relu rescale, all folded into the single final
per-partition multiply (rstd/65536) -- exact in binary. The augmented matmul
and stats stay bf16. Simulated output error ~7.5e-3 vs the 2e-2 gate.

fp8 enables DoubleRow perf mode: both k-chunks contract in ONE matmul
(2 MACs/cell/cycle), halving mm1/mm2 PE cycles.

Sharding: 4 row-groups x 2 HID-halves. DMAs: 2 blobs on Sync + 2 on Scalar
(HWDGE only -- SWDGE adds ~2us latency), ordered so the stats path (xr) and
mm1 operands land earliest. Outputs leave per row-tile on Sync/Scalar.
"""

import sys

if '/opt/trn_rl_repo' not in sys.path:
    sys.path.insert(0, '/opt/trn_rl_repo')

import ml_dtypes
import numpy as np

import concourse.bass as bass  # noqa: F401
import concourse.tile as tile
from concourse import bacc, mybir
from concourse.bass_utils import run_bass_kernel_spmd

B, T, C = 2, 512, 256
HID = 4 * C
EPS = 1e-5
N_CORES = 8
N_GROUPS = 4                       # row groups
ROWS = (B * T) // N_GROUPS         # 256 rows per core
RT = ROWS // 128                   # 2 row tiles per core
HH = HID // 2                      # 512-wide hidden half per core
KC = C // 128                      # 2 k-subtiles over C
KH = HH // 128                     # 4 m-chunks over the half

F32 = mybir.dt.float32
BF16 = mybir.dt.bfloat16
FP8 = mybir.dt.float8e4
BF16_NP = ml_dtypes.bfloat16
FP8_NP = ml_dtypes.float8_e4m3

S1 = 1024.0                        # w1 scale (power of 2)
S2 = 1024.0                        # w2 scale
SR = 1.0 / 16.0                    # relu output rescale
STOT = S1 * S2 * SR                # folded into the final rstd multiply


def _build_nc():
    nc = bacc.Bacc("TRN2", target_bir_lowering=False, debug=False,
                   num_devices=N_CORES)

    # xr rows (bf16) + identity for the tiny stats transpose
    inxr_d = nc.declare_dram_parameter("inxr", [128, RT * C + 128], BF16,
                                       isOutput=False)
    # fp8 blob, per k-plane: [xt_k (256) | w1t_k (512)]
    inw1x_d = nc.declare_dram_parameter("inw1x", [128, KC, 768], FP8,
                                        isOutput=False)
    inw2_d = nc.declare_dram_parameter("inw2", [128, KH, C], FP8,
                                       isOutput=False)
    augw_d = nc.declare_dram_parameter("augw", [2, HH], BF16, isOutput=False)
    y_d = nc.declare_dram_parameter("y", [128, RT * C], BF16, isOutput=True)

    DR = mybir.MatmulPerfMode.DoubleRow

    with tile.TileContext(nc) as tc:
        with (
            tc.tile_pool(name="acts", bufs=1) as acts,
            tc.tile_pool(name="stats", bufs=2) as stats,
            tc.tile_pool(name="ptrans", bufs=2, space="PSUM") as ptrans,
            tc.tile_pool(name="pmm1", bufs=4, space="PSUM") as pmm1,
            tc.tile_pool(name="pmm2", bufs=2, space="PSUM") as pmm2,
        ):
            # ---- input DMAs: two HWDGE queues ----
            inxr_sb = acts.tile([128, RT * C + 128], BF16)
            nc.sync.dma_start(out=inxr_sb, in_=inxr_d.ap())
            inw1x_sb = acts.tile([128, KC, 768], FP8)
            nc.sync.dma_start(out=inw1x_sb, in_=inw1x_d.ap())

            inw2_sb = acts.tile([128, KH, C], FP8)
            nc.scalar.dma_start(out=inw2_sb, in_=inw2_d.ap())
            augw_sb = acts.tile([2, HH], BF16)
            nc.scalar.dma_start(out=augw_sb, in_=augw_d.ap())

            eps_t = acts.tile([128, 1], F32)
            nc.vector.memset(eps_t, np.float32(EPS))

            ident = inxr_sb[:, RT * C:RT * C + 128]

            # ---- stats per row tile ----
            # aug_rhs[0,t] = mu[t] (bf16); aug_rhs[1,t] = sqrt(var[t]+eps)
            aug_rhs = acts.tile([2, ROWS], BF16)
            rstd_s = []
            stgs = []
            for r in range(RT):
                xr = inxr_sb[:, r * C:(r + 1) * C]
                bn6 = stats.tile([128, 6], F32, tag="bn6")
                nc.vector.bn_stats(out=bn6, in_=xr)
                mv = stats.tile([128, 2], F32, tag="mv")
                nc.vector.bn_aggr(out=mv, in_=bn6)
                sqv = stats.tile([128, 1], F32, tag="sqv")
                nc.scalar.activation(out=sqv, in_=mv[:, 1:2],
                                     func=mybir.ActivationFunctionType.Sqrt,
                                     bias=eps_t, scale=1.0)
                stg = stats.tile([128, 2], BF16, tag="stg")
                nc.vector.tensor_copy(out=stg[:, 0:1], in_=mv[:, 0:1])
                nc.vector.tensor_copy(out=stg[:, 1:2], in_=sqv)
                stgs.append(stg)
                # rstd/STOT for the final scale
                rstd = stats.tile([128, 1], F32, tag="rstd")
                nc.vector.reciprocal(out=rstd, in_=sqv)
                rs = stats.tile([128, 1], F32, tag="rs")
                nc.vector.tensor_scalar_mul(rs, rstd, 1.0 / STOT)
                rstd_s.append(rs)

            # ---- PE: raw DR matmuls interleaved with the stat transposes ----
            ps1 = [pmm1.tile([128, ROWS], F32, tag=f"ps1_{i}", bufs=1,
                             name=f"ps1_{i}")
                   for i in range(KH)]
            pts = []
            for mc in range(KH):
                nc.tensor.matmul(
                    ps1[mc],
                    lhsT=inw1x_sb[:, :, 256 + mc * 128:256 + (mc + 1) * 128],
                    rhs=inw1x_sb[:, :, 0:256],
                    start=True, stop=False,
                    perf_mode=DR,
                )
                if mc < RT:
                    pt = ptrans.tile([2, 128], BF16)
                    nc.tensor.transpose(pt, stgs[mc], ident)
                    pts.append(pt)
                    nc.vector.tensor_copy(
                        out=aug_rhs[:, mc * 128:(mc + 1) * 128], in_=pt)

            # ---- aug matmul (bf16) + relu (alternating engines) ----
            relu1 = acts.tile([128, KH, ROWS], FP8)
            for mc in range(KH):
                nc.tensor.matmul(
                    ps1[mc],
                    lhsT=augw_sb[:, mc * 128:(mc + 1) * 128],
                    rhs=aug_rhs,
                    start=False, stop=True,
                )
                if mc % 2 == 0:
                    nc.scalar.activation(
                        out=relu1[:, mc, :], in_=ps1[mc],
                        func=mybir.ActivationFunctionType.Relu,
                        bias=0.0, scale=float(SR))
                else:
                    nc.vector.tensor_scalar(
                        out=relu1[:, mc, :], in0=ps1[mc],
                        scalar1=0.0, scalar2=float(SR),
                        op0=mybir.AluOpType.max,
                        op1=mybir.AluOpType.mult)

            # ---- mm2 (fp8 DR) + final rstd/STOT scale per row tile ----
            y_sb = acts.tile([128, RT, C], BF16)
            for r in range(RT):
                po = pmm2.tile([128, C], F32)
                for j in range(KH // 2):
                    nc.tensor.matmul(
                        po,
                        lhsT=relu1[:, 2 * j:2 * j + 2, r * 128:(r + 1) * 128],
                        rhs=inw2_sb[:, 2 * j:2 * j + 2, :],
                        start=(j == 0), stop=(j == KH // 2 - 1),
                        perf_mode=DR,
                    )
                if r == 0:
                    nc.vector.tensor_scalar_mul(y_sb[:, 0, :], po, rstd_s[0])
                    nc.sync.dma_start(out=y_d.ap()[:, :C], in_=y_sb[:, 0, :])
                else:
                    nc.scalar.activation(
                        out=y_sb[:, 1, :], in_=po,
                        func=mybir.ActivationFunctionType.Copy,
                        bias=0.0, scale=rstd_s[1])
                    nc.scalar.dma_start(out=y_d.ap()[:, C:],
                                        in_=y_sb[:, 1, :])

    nc.finalize()
    return nc


_NC_CACHE = None


def _get_nc():
    global _NC_CACHE
    if _NC_CACHE is None:
        _NC_CACHE = _build_nc()
    return _NC_CACHE


def _q8(a, scale):
    s = np.asarray(a, dtype=np.float64) * scale
    s = np.clip(s, -240.0, 240.0)
    return s.astype(np.float32).astype(FP8_NP)


def _pack_inputs(x, bp, g2, b2, W1, bf1, W2):
    x1 = (np.asarray(x, dtype=np.float64).reshape(B * T, C)
          + np.asarray(bp, dtype=np.float64))
    x1_f32 = x1.astype(np.float32)
    x1_bf = x1_f32.astype(BF16_NP)
    x1_f8 = x1_bf.astype(np.float32).astype(FP8_NP)   # |x| << 240, no clip

    w1t_eff = (np.asarray(W1).astype(np.float64).T
               * np.asarray(g2).astype(np.float64)[:, None])      # [C, HID]
    w1t_f8 = _q8(w1t_eff, S1)
    bf1_eff = (np.asarray(bf1).astype(np.float64)
               + np.asarray(b2).astype(np.float64)
               @ np.asarray(W1).astype(np.float64).T)             # [HID]
    # aug row0 = -sum_c of the *scaled fp8 weights actually used*
    s1_scaled = w1t_f8.astype(np.float64).sum(axis=0)             # S1-scaled
    aug0 = (-s1_scaled).astype(np.float32).astype(BF16_NP)
    aug1 = (bf1_eff * S1).astype(np.float32).astype(BF16_NP)
    w2t_f8 = _q8(np.asarray(W2, dtype=np.float64).T, S2)          # [HID, C]
    ident = np.eye(128, dtype=np.float32).astype(BF16_NP)

    in_maps = []
    for c in range(N_CORES):
        g, hf = c // 2, c % 2
        xg_bf = x1_bf[g * ROWS:(g + 1) * ROWS]                    # [256, C]
        xg_f8 = x1_f8[g * ROWS:(g + 1) * ROWS]

        inxr = np.empty((128, RT * C + 128), dtype=BF16_NP)
        for r in range(RT):
            inxr[:, r * C:(r + 1) * C] = xg_bf[r * 128:(r + 1) * 128, :]
        inxr[:, RT * C:] = ident

        inw1x = np.empty((128, KC, 768), dtype=FP8_NP)
        w1h = w1t_f8[:, hf * HH:(hf + 1) * HH]                    # [C, HH]
        for k in range(KC):
            inw1x[:, k, 0:256] = xg_f8[:, k * 128:(k + 1) * 128].T
            inw1x[:, k, 256:768] = w1h[k * 128:(k + 1) * 128, :]

        inw2 = np.empty((128, KH, C), dtype=FP8_NP)
        w2h = w2t_f8[hf * HH:(hf + 1) * HH]                       # [HH, C]
        for mc in range(KH):
            inw2[:, mc, :] = w2h[mc * 128:(mc + 1) * 128, :]

        augw = np.empty((2, HH), dtype=BF16_NP)
        augw[0] = aug0[hf * HH:(hf + 1) * HH]
        augw[1] = aug1[hf * HH:(hf + 1) * HH]

        in_maps.append({"inxr": inxr, "inw1x": inw1x, "inw2": inw2,
                        "augw": augw})
    return in_maps, x1_f32


def _make_in_maps(x, bp, g2, b2, W1, bf1, W2):
    in_maps, _ = _pack_inputs(x, bp, g2, b2, W1, bf1, W2)
    return in_maps


def kernel(x, Wt, Wp, bp, g1, b1, g2, b2, W1, bf1, W2, bf2):
    in_maps, x1_f32 = _pack_inputs(x, bp, g2, b2, W1, bf1, W2)
    nc = _get_nc()
    res = run_bass_kernel_spmd(nc, in_maps, list(range(N_CORES)))

    out = x1_f32.copy()                                       # residual x+bp
    for g in range(N_GROUPS):
        for hf in range(2):
            y = np.asarray(res.results[2 * g + hf]["y"]).astype(np.float32)
            for r in range(RT):
                out[g * ROWS + r * 128:g * ROWS + (r + 1) * 128, :] += \
                    y[:, r * C:(r + 1) * C]
    out = out + np.asarray(bf2, dtype=np.float32)
    return out.reshape(B, T, C).astype(np.float32)
